# revision 1
# baseline (speedup 1.0000x reference)
"""Correlation cost-volume kernel (max_displacement=4) for 8 Trainium2 cores.

Problem: in1, in2: [B=8, C=256, H=128, W=128] f32.
out[b, dy*9+dx, h, w] = sum_c in1[b,c,h,w] * pad(in2)[b, c, h+dy, w+dx]
(pad = 4 zeros on each spatial side), output [8, 81, 128, 128] f32.

Strategy (data-parallel, one batch sample per core):
  For each output row h and each dy (9 values), the TensorEngine computes the
  row-gram  G[w, w'] = sum_c in1[c,h,w] * in2pad[c,h+dy,w']  ([128 x 136],
  contracting C=256 as two K=128 tiles accumulated in PSUM f32; operands are
  bf16, N=408 per matmul).  The 9 needed entries per w are the near-diagonals
  G[w, w+dx], dx in 0..8 — a shear, which no on-chip engine or rectangular
  DMA access pattern can extract, and descriptor-unrolled skewed DMAs explode
  neuronxcc compile time.  So the device emits the full row-grams as its
  output (rectangular, per-partition-contiguous DMAs) and the host slices the
  9 diagonals per row with numpy stride tricks.
"""

import ml_dtypes
import numpy as np

import concourse.bass as bass
import concourse.bacc as bacc
import concourse.mybir as mybir
from concourse.bass_utils import run_bass_kernel_spmd
from concourse.tile import TileContext

B, C, H, W = 8, 256, 128, 128
D = 4
ND = 2 * D + 1  # 9 displacements per axis
WP = W + 2 * D  # 136 padded width / # of gram columns
KT = C // 128  # 2 contraction tiles
GD = 3  # dy rows per PSUM bank (N = 3*136 = 408 <= 512)
NG = ND // GD  # 3 PSUM banks per output row
HG = 2  # h rows per gram-output DMA
AH = 8  # h rows per in1 load DMA

OUT_DT = mybir.dt.bfloat16  # gram output dtype (float32 | bfloat16)
_OUT_NP = ml_dtypes.bfloat16 if OUT_DT == mybir.dt.bfloat16 else np.float32

_CACHED_NC = None


def _build_nc():
    bf16 = mybir.dt.bfloat16

    nc = bacc.Bacc()
    # in1 as [c][h][kt][w]; in2 zero-padded to [kt][c][hp][wp]; both bf16
    in1_t = nc.declare_dram_parameter("in1_t", [128, H, KT, W], bf16, isOutput=False)
    in2_p = nc.declare_dram_parameter("in2_p", [KT, 128, WP, WP], bf16, isOutput=False)
    # full row-grams, laid out so each partition (w) writes one contiguous run
    out_g = nc.declare_dram_parameter(
        "out_g", [H // HG, W, HG, ND, WP], OUT_DT, isOutput=True
    )

    with TileContext(nc) as tc:
        with (
            tc.tile_pool(name="bpool", bufs=1) as bpool,
            tc.tile_pool(name="apool", bufs=3) as apool,
            tc.tile_pool(name="spool", bufs=3) as spool,
            tc.tile_pool(name="psum", bufs=2, space="PSUM") as ppool,
        ):
            # whole padded in2 sample resident in SBUF: 2*136*136*2B = 72KB/partition
            # single DMA per kt — more DMAs here means more distinct DMA-queue
            # semaphores the first matmuls must wait on, and walrus rejects
            # matmuls with too many sync-wait commands.
            b_s = bpool.tile([128, KT, WP, WP], bf16)
            for kt in range(KT):
                nc.sync.dma_start(out=b_s[:, kt], in_=in2_p[kt])

            st = None
            for h in range(H):
                if h % AH == 0:
                    a_t = apool.tile([128, AH, KT, W], bf16)
                    nc.sync.dma_start(out=a_t, in_=in1_t[:, h : h + AH])
                if h % HG == 0:
                    st = spool.tile([128, HG, ND, WP], OUT_DT)

                pss = [
                    ppool.tile([128, GD * WP], mybir.dt.float32, name=f"ps{g}", tag=f"ps{g}")
                    for g in range(NG)
                ]
                for kt in range(KT):
                    lhsT = a_t[:, h % AH, kt, :]
                    for g in range(NG):
                        rhs = b_s[:, kt, h + GD * g : h + GD * g + GD, :]
                        nc.tensor.matmul(
                            pss[g],
                            lhsT,
                            rhs,
                            start=(kt == 0),
                            stop=(kt == KT - 1),
                        )
                for g in range(NG):
                    nc.any.tensor_copy(
                        st[:, h % HG, GD * g : GD * g + GD, :],
                        pss[g].rearrange("w (d p) -> w d p", d=GD),
                    )
                if h % HG == HG - 1:
                    # partition w -> contiguous HG*ND*WP run in DRAM
                    nc.sync.dma_start(
                        out=out_g[h // HG].rearrange("w g d p -> w (g d p)"),
                        in_=st.rearrange("w g d p -> w (g d p)"),
                    )

    # Run the bacc passes (move_matmul_waits_to_ldweights /
    # generate_event_semaphores) that enforce the 1-wait-per-instruction HW
    # constraint.  The native run path calls this inside run_bass_kernel_spmd;
    # the axon/bass2jax path serializes nc without it and walrus then rejects
    # matmuls carrying two sync waits.
    nc.compile()
    return nc


def _get_nc():
    global _CACHED_NC
    if _CACHED_NC is None:
        _CACHED_NC = _build_nc()
    return _CACHED_NC


def _make_in_maps(in1: np.ndarray, in2: np.ndarray):
    in_maps = []
    for b in range(B):
        # [C,H,W] -> [c(128), H, kt, W] so one DMA per h-block is contiguous
        a = np.ascontiguousarray(
            in1[b].astype(ml_dtypes.bfloat16).reshape(KT, 128, H, W).transpose(1, 2, 0, 3)
        )
        p = np.zeros((C, WP, WP), ml_dtypes.bfloat16)
        p[:, D : D + H, D : D + W] = in2[b].astype(ml_dtypes.bfloat16)
        in_maps.append({"in1_t": a, "in2_p": p.reshape(KT, 128, WP, WP)})
    return in_maps


def _extract_band(g: np.ndarray) -> np.ndarray:
    """[H//HG, W, HG, ND, WP] full row-grams -> [81, H, W] cost volume."""
    # -> [H, ND, W, WP]
    g2 = np.ascontiguousarray(
        g.reshape(H // HG, W, HG, ND, WP).transpose(0, 2, 3, 1, 4).astype(np.float32)
    ).reshape(H, ND, W, WP)
    # band: out[h, dy, w, dx] = g2[h, dy, w, w + dx]
    sw = np.lib.stride_tricks.sliding_window_view(g2, ND, axis=3)  # [H,ND,W,128,ND]
    iw = np.arange(W)
    band = sw[:, :, iw, iw, :]  # [H, ND, W, ND]
    return np.ascontiguousarray(band.transpose(1, 3, 0, 2)).reshape(ND * ND, H, W)


def kernel(**inputs) -> np.ndarray:
    in1 = np.ascontiguousarray(np.asarray(inputs["in1"], dtype=np.float32))
    in2 = np.ascontiguousarray(np.asarray(inputs["in2"], dtype=np.float32))
    assert in1.shape == (B, C, H, W) and in2.shape == (B, C, H, W)

    nc = _get_nc()
    in_maps = _make_in_maps(in1, in2)
    res = run_bass_kernel_spmd(nc, in_maps, list(range(B)))

    outs = [_extract_band(np.asarray(res.results[b]["out_g"])) for b in range(B)]
    return np.stack(outs).astype(np.float32)



# revision 7
# speedup vs baseline: 1.8407x; 1.8407x over previous
"""Correlation cost-volume kernel (max_displacement=4) for 8 Trainium2 cores.

Problem: in1, in2: [B=8, C=256, H=128, W=128] f32.
out[b, dy*9+dx, h, w] = sum_c in1[b,c,h,w] * pad(in2)[b, c, h+dy, w+dx]
(pad = 4 zeros on each spatial side), output [8, 81, 128, 128] f32.

Strategy (data-parallel, one batch sample per core):
  The needed outputs are a band-of-band of the cross-gram
  G[(h,w'),(r,w2)] = sum_c in1p[c,h,w'] * in2p[c,r,w2]  (useful iff
  r-h in [0,9) and w2-w' in [-8,1)).  Tile it into (TH=8 h-rows x TW=8
  w2-cols) chunks: one matmul per chunk takes weights = in1p[c, 16 w'
  cols x 8 h rows] (M=128 = exactly the PE width; in1 is host-transposed
  to [kt,c,hc,w',h] so the block is contiguous - the BIR verifier only
  allows one free dim on the stationary operand) against moving
  in2p[c, 16 r rows, 8 w2 cols] (N=128), covering all 81 (dy,dx) pairs
  for its 8x8 output block with only ~3.2x padding waste (vs 15.1x for
  full 136-wide row grams).  C=256 contracts as 2 K=128 matmuls
  accumulated in PSUM.  Chunks ship to DRAM as dense [128,128] bf16
  blocks (4 chunks batched per PSUM bank / copy / DMA); the host slices
  the 81 (dy,dx) diagonal planes with one vectorized fancy-index per
  plane (the shear couples output partition to free offset, which no
  on-chip engine or rectangular DMA access pattern can express).
"""

import ml_dtypes
import numpy as np

import concourse.bass as bass
import concourse.bacc as bacc
import concourse.mybir as mybir
from concourse.bass_utils import run_bass_kernel_spmd
from concourse.tile import TileContext

B, C, H, W = 8, 256, 128, 128
D = 4
ND = 2 * D + 1  # 9 displacements per axis
KT = C // 128  # 2 contraction tiles
WP = W + 2 * D  # 136 padded in2 width (w2 = w + dx space)
WI = W + 16  # 144: in1 padded by 8 on each side (w' = w space, offset 8)
TH = 8  # output h rows per chunk
RH = TH + ND - 1  # 16 in2p rows per chunk (r = h + dy)
TW = 8  # w2 cols per chunk
RW = TW + ND - 1  # 16 in1 cols per chunk (w' = w2 - dx, dx in [0,9))
NHC = H // TH  # 16 h-chunks
NWC = WP // TW  # 17 w2-chunks
GRP = 4  # w-chunks per PSUM bank (4*128 = 512 f32 = one 2KB bank)
NB2 = 8  # in2 row-band loads of 17 rows each

_CACHED_NC = None


def _build_nc():
    bf16 = mybir.dt.bfloat16
    f32 = mybir.dt.float32

    nc = bacc.Bacc()
    # in1 padded to 144 w' cols (zeros at [0,8) and [136,144)) and laid out
    # [kt, c, hc, w', h_loc]; in2 zero-padded to 136x136 in natural row
    # order; both bf16, split into KT=2 blocks of 128 channels.
    in1_t = nc.declare_dram_parameter("in1_t", [KT, 128, NHC, WI, TH], bf16, isOutput=False)
    in2_p = nc.declare_dram_parameter("in2_p", [KT, 128, WP, WP], bf16, isOutput=False)
    # dense gram chunks: [hc][m=(w'_loc 16, h_loc 8)][wc][n=(r_loc 16, w2_loc 8)]
    out_g = nc.declare_dram_parameter("out_g", [NHC, 128, NWC, RH * TW], bf16, isOutput=True)

    wgroups = [list(range(g, min(g + GRP, NWC))) for g in range(0, NWC, GRP)]

    with TileContext(nc) as tc:
        with (
            tc.tile_pool(name="bpool", bufs=1) as bpool,
            tc.tile_pool(name="apool", bufs=1) as apool,
            tc.tile_pool(name="spool", bufs=6) as spool,
            tc.tile_pool(name="psum", bufs=8, space="PSUM") as ppool,
        ):
            # whole padded sample resident in SBUF: in2p 72.3KB + in1p 72KB
            # per partition.  Loads are issued in row bands, interleaved so
            # early h-chunks can start while later rows are still in flight.
            b_s = bpool.tile([128, KT, WP, WP], bf16)
            a_s = apool.tile([128, KT, NHC, WI, TH], bf16)
            for i in range(NB2):
                r0 = 17 * i
                for kt in range(KT):
                    nc.sync.dma_start(
                        out=b_s[:, kt, r0 : r0 + 17, :], in_=in2_p[kt, :, r0 : r0 + 17, :]
                    )
                for hcl in (2 * i, 2 * i + 1):
                    for kt in range(KT):
                        nc.sync.dma_start(
                            out=a_s[:, kt, hcl], in_=in1_t[kt, :, hcl]
                        )

            for hc in range(NHC):
                h0 = TH * hc
                for wg, wcs in enumerate(wgroups):
                    ps = ppool.tile([128, GRP * RH * TW], f32, name=f"ps{wg}", tag="ps")
                    for j, wc in enumerate(wcs):
                        w0 = TW * wc
                        for kt in range(KT):
                            nc.tensor.matmul(
                                ps[:, 128 * j : 128 * j + 128],
                                a_s[:, kt, hc, w0 : w0 + RW, :],
                                b_s[:, kt, h0 : h0 + RH, w0 : w0 + TW],
                                start=(kt == 0),
                                stop=(kt == KT - 1),
                            )
                    ncol = len(wcs) * RH * TW
                    st = spool.tile([128, GRP * RH * TW], bf16)
                    nc.any.tensor_copy(st[:, :ncol], ps[:, :ncol])
                    nc.sync.dma_start(
                        out=out_g[hc, :, GRP * wg : GRP * wg + len(wcs), :].rearrange(
                            "p g n -> p (g n)"
                        ),
                        in_=st[:, :ncol],
                    )

    # Run the bacc passes (move_matmul_waits_to_ldweights /
    # generate_event_semaphores) that enforce the 1-wait-per-instruction HW
    # constraint.  The native run path calls this inside run_bass_kernel_spmd;
    # the axon/bass2jax path serializes nc without it and walrus then rejects
    # matmuls carrying two sync waits.
    nc.compile()
    return nc


def _get_nc():
    global _CACHED_NC
    if _CACHED_NC is None:
        _CACHED_NC = _build_nc()
    return _CACHED_NC


def _make_in_maps(in1: np.ndarray, in2: np.ndarray):
    in_maps = []
    for b in range(B):
        a = np.zeros((KT, 128, NHC, WI, TH), ml_dtypes.bfloat16)
        # [kt, c, hc, h_loc, w] -> [kt, c, hc, w(+8), h_loc]
        a[:, :, :, 8 : 8 + W, :] = (
            in1[b].astype(ml_dtypes.bfloat16).reshape(KT, 128, NHC, TH, W)
        ).transpose(0, 1, 2, 4, 3)
        p = np.zeros((KT, 128, WP, WP), ml_dtypes.bfloat16)
        p[:, :, D : D + H, D : D + W] = in2[b].astype(ml_dtypes.bfloat16).reshape(
            KT, 128, H, W
        )
        in_maps.append({"in1_t": a, "in2_p": p})
    return in_maps


def _extract_band(g: np.ndarray) -> np.ndarray:
    """[NHC, 128, NWC, 128] dense gram chunks -> [81, H, W] cost volume."""
    rf = np.ascontiguousarray(g).astype(np.float32)
    # [hc, w'_loc, h_loc, wc, r_loc, w2_loc]
    r6 = rf.reshape(NHC, RW, TH, NWC, RH, TW)
    hl = np.arange(TH)[:, None]  # (8,1)
    w2 = np.arange(TW)[None, :]  # (1,8)
    out = np.empty((ND * ND, H, W), np.float32)
    for dy in range(ND):
        for dx in range(ND):
            # advanced indices at axes 1,2,4,5 -> result (8,8,NHC,NWC)
            t = r6[:, w2 + 8 - dx, hl, :, hl + dy, w2]
            # (h_loc, w2_loc, hc, wc) -> (hc, h_loc, wc, w2_loc) = (h, w+dx)
            wfull = t.transpose(2, 0, 3, 1).reshape(H, NWC * TW)
            out[dy * ND + dx] = wfull[:, dx : dx + W]
    return out


def kernel(**inputs) -> np.ndarray:
    in1 = np.ascontiguousarray(np.asarray(inputs["in1"], dtype=np.float32))
    in2 = np.ascontiguousarray(np.asarray(inputs["in2"], dtype=np.float32))
    assert in1.shape == (B, C, H, W) and in2.shape == (B, C, H, W)

    nc = _get_nc()
    in_maps = _make_in_maps(in1, in2)
    res = run_bass_kernel_spmd(nc, in_maps, list(range(B)))

    outs = [_extract_band(np.asarray(res.results[b]["out_g"])) for b in range(B)]
    return np.stack(outs).astype(np.float32)


# revision 9
# speedup vs baseline: 2.4384x; 1.3247x over previous
"""Correlation cost-volume kernel (max_displacement=4) for 8 Trainium2 cores.

Problem: in1, in2: [B=8, C=256, H=128, W=128] f32.
out[b, dy*9+dx, h, w] = sum_c in1[b,c,h,w] * pad(in2)[b, c, h+dy, w+dx]
(pad = 4 zeros on each spatial side), output [8, 81, 128, 128] f32.

Strategy (data-parallel, one batch sample per core):
  The needed outputs are a band-of-band of the cross-gram
  G[(h,w'),(r,w2)] = sum_c in1p[c,h,w'] * in2p[c,r,w2]  (useful iff
  r-h in [0,9) and w2-w' in [-8,1)).  Tile it into (TH=8 h-rows x TW=8
  w2-cols) chunks: one matmul per chunk takes weights = in1p[c, 16 w'
  cols x 8 h rows] (M=128 = exactly the PE width; in1 is host-transposed
  to [kt,c,hc,w',h] so the block is contiguous - the BIR verifier only
  allows one free dim on the stationary operand) against moving
  in2p[c, 16 r rows, 8 w2 cols] (N=128), covering all 81 (dy,dx) pairs
  for its 8x8 output block with only ~3.2x padding waste (vs 15.1x for
  full 136-wide row grams).  C=256 contracts as 2 K=128 matmuls
  accumulated in PSUM.  Chunks ship to DRAM as dense [128,128] bf16
  blocks (4 chunks batched per PSUM bank / copy / DMA); the host slices
  the 81 (dy,dx) diagonal planes with one vectorized fancy-index per
  plane (the shear couples output partition to free offset, which no
  on-chip engine or rectangular DMA access pattern can express).
"""

import ml_dtypes
import numpy as np

import concourse.bass as bass
import concourse.bacc as bacc
import concourse.mybir as mybir
from concourse.bass_utils import run_bass_kernel_spmd
from concourse.tile import TileContext

B, C, H, W = 8, 256, 128, 128
D = 4
ND = 2 * D + 1  # 9 displacements per axis
KT = C // 128  # 2 contraction tiles
WP = W + 2 * D  # 136 padded in2 width (w2 = w + dx space)
WI = W + 16  # 144: in1 padded by 8 on each side (w' = w space, offset 8)
TH = 8  # output h rows per chunk
RH = TH + ND - 1  # 16 in2p rows per chunk (r = h + dy)
TW = 8  # w2 cols per chunk
RW = TW + ND - 1  # 16 in1 cols per chunk (w' = w2 - dx, dx in [0,9))
NHC = H // TH  # 16 h-chunks
NWC = WP // TW  # 17 w2-chunks
GRP = 4  # w-chunks per PSUM bank (4*128 = 512 f32 = one 2KB bank)
NB2 = 4  # in2 row-band loads of 34 rows each (DGE spreads packets over all engines)
HCG = 4  # h-chunks per in1 load

_CACHED_NC = None


def _build_nc():
    bf16 = mybir.dt.bfloat16
    f32 = mybir.dt.float32

    nc = bacc.Bacc()
    # in1 padded to 144 w' cols (zeros at [0,8) and [136,144)) and laid out
    # [kt, c, hc, w', h_loc]; in2 zero-padded to 136x136 in natural row
    # order; both bf16, split into KT=2 blocks of 128 channels.
    in1_t = nc.declare_dram_parameter("in1_t", [KT, 128, NHC, WI, TH], bf16, isOutput=False)
    in2_p = nc.declare_dram_parameter("in2_p", [KT, 128, WP, WP], bf16, isOutput=False)
    # dense gram chunks: [hc][m=(w'_loc 16, h_loc 8)][wc][n=(r_loc 16, w2_loc 8)]
    out_g = nc.declare_dram_parameter("out_g", [NHC, 128, NWC, RH * TW], bf16, isOutput=True)

    wgroups = [list(range(g, min(g + GRP, NWC))) for g in range(0, NWC, GRP)]

    with TileContext(nc) as tc:
        with (
            tc.tile_pool(name="bpool", bufs=1) as bpool,
            tc.tile_pool(name="apool", bufs=1) as apool,
            tc.tile_pool(name="spool", bufs=6) as spool,
            tc.tile_pool(name="psum", bufs=8, space="PSUM") as ppool,
        ):
            # whole padded sample resident in SBUF: in2p 72.3KB + in1p 72KB
            # per partition.  Loads are issued in row bands, interleaved so
            # early h-chunks can start while later rows are still in flight.
            b_s = bpool.tile([128, KT, WP, WP], bf16)
            a_s = apool.tile([128, KT, NHC, WI, TH], bf16)
            # 16 load DMAs of ~1.18MB each, in need-order; the DGE spreads
            # each instruction's packets across all 16 DMA engines, and each
            # engine drains its queue in order, so loads complete roughly in
            # issue order at full aggregate bandwidth.
            rb = WP // NB2  # 34
            for i in range(NB2):
                for kt in range(KT):
                    nc.sync.dma_start(
                        out=b_s[:, kt, rb * i : rb * i + rb, :],
                        in_=in2_p[kt, :, rb * i : rb * i + rb, :],
                    )
                for kt in range(KT):
                    nc.sync.dma_start(
                        out=a_s[:, kt, HCG * i : HCG * i + HCG],
                        in_=in1_t[kt, :, HCG * i : HCG * i + HCG],
                    )

            for hc in range(NHC):
                h0 = TH * hc
                st = spool.tile([128, NWC * RH * TW], bf16)
                for wg, wcs in enumerate(wgroups):
                    ps = ppool.tile([128, GRP * RH * TW], f32, name=f"ps{wg}", tag="ps")
                    for j, wc in enumerate(wcs):
                        w0 = TW * wc
                        for kt in range(KT):
                            nc.tensor.matmul(
                                ps[:, 128 * j : 128 * j + 128],
                                a_s[:, kt, hc, w0 : w0 + RW, :],
                                b_s[:, kt, h0 : h0 + RH, w0 : w0 + TW],
                                start=(kt == 0),
                                stop=(kt == KT - 1),
                            )
                    ncol = len(wcs) * RH * TW
                    c0 = GRP * RH * TW * wg
                    nc.any.tensor_copy(st[:, c0 : c0 + ncol], ps[:, :ncol])
                nc.sync.dma_start(
                    out=out_g[hc].rearrange("p w n -> p (w n)"),
                    in_=st,
                )

    # Run the bacc passes (move_matmul_waits_to_ldweights /
    # generate_event_semaphores) that enforce the 1-wait-per-instruction HW
    # constraint.  The native run path calls this inside run_bass_kernel_spmd;
    # the axon/bass2jax path serializes nc without it and walrus then rejects
    # matmuls carrying two sync waits.
    nc.compile()
    return nc


def _get_nc():
    global _CACHED_NC
    if _CACHED_NC is None:
        _CACHED_NC = _build_nc()
    return _CACHED_NC


def _make_in_maps(in1: np.ndarray, in2: np.ndarray):
    in_maps = []
    for b in range(B):
        a = np.zeros((KT, 128, NHC, WI, TH), ml_dtypes.bfloat16)
        # [kt, c, hc, h_loc, w] -> [kt, c, hc, w(+8), h_loc]
        a[:, :, :, 8 : 8 + W, :] = (
            in1[b].astype(ml_dtypes.bfloat16).reshape(KT, 128, NHC, TH, W)
        ).transpose(0, 1, 2, 4, 3)
        p = np.zeros((KT, 128, WP, WP), ml_dtypes.bfloat16)
        p[:, :, D : D + H, D : D + W] = in2[b].astype(ml_dtypes.bfloat16).reshape(
            KT, 128, H, W
        )
        in_maps.append({"in1_t": a, "in2_p": p})
    return in_maps


def _extract_band(g: np.ndarray) -> np.ndarray:
    """[NHC, 128, NWC, 128] dense gram chunks -> [81, H, W] cost volume."""
    rf = np.ascontiguousarray(g).astype(np.float32)
    # [hc, w'_loc, h_loc, wc, r_loc, w2_loc]
    r6 = rf.reshape(NHC, RW, TH, NWC, RH, TW)
    hl = np.arange(TH)[:, None]  # (8,1)
    w2 = np.arange(TW)[None, :]  # (1,8)
    out = np.empty((ND * ND, H, W), np.float32)
    for dy in range(ND):
        for dx in range(ND):
            # advanced indices at axes 1,2,4,5 -> result (8,8,NHC,NWC)
            t = r6[:, w2 + 8 - dx, hl, :, hl + dy, w2]
            # (h_loc, w2_loc, hc, wc) -> (hc, h_loc, wc, w2_loc) = (h, w+dx)
            wfull = t.transpose(2, 0, 3, 1).reshape(H, NWC * TW)
            out[dy * ND + dx] = wfull[:, dx : dx + W]
    return out


def kernel(**inputs) -> np.ndarray:
    in1 = np.ascontiguousarray(np.asarray(inputs["in1"], dtype=np.float32))
    in2 = np.ascontiguousarray(np.asarray(inputs["in2"], dtype=np.float32))
    assert in1.shape == (B, C, H, W) and in2.shape == (B, C, H, W)

    nc = _get_nc()
    in_maps = _make_in_maps(in1, in2)
    res = run_bass_kernel_spmd(nc, in_maps, list(range(B)))

    outs = [_extract_band(np.asarray(res.results[b]["out_g"])) for b in range(B)]
    return np.stack(outs).astype(np.float32)


# revision 13
# speedup vs baseline: 2.8703x; 1.1771x over previous
"""Correlation cost-volume kernel (max_displacement=4) for 8 Trainium2 cores.

Problem: in1, in2: [B=8, C=256, H=128, W=128] f32.
out[b, dy*9+dx, h, w] = sum_c in1[b,c,h,w] * pad(in2)[b, c, h+dy, w+dx]
(pad = 4 zeros on each spatial side), output [8, 81, 128, 128] f32.

Strategy (data-parallel, one batch sample per core):
  The needed outputs are a band-of-band of the cross-gram
  G[(h,w'),(r,w2)] = sum_c in1p[c,h,w'] * in2p[c,r,w2]  (useful iff
  r-h in [0,9) and w2-w' in [-8,1)).  Tile it into (TH=8 h-rows x TW=8
  w2-cols) chunks: one matmul per chunk takes weights = in1p[c, 16 w'
  cols x 8 h rows] (M=128 = exactly the PE width; in1 is host-transposed
  to [kt,c,hc,w',h] so the block is contiguous - the BIR verifier only
  allows one free dim on the stationary operand) against moving
  in2p[c, 16 r rows, 8 w2 cols] (N=128), covering all 81 (dy,dx) pairs
  for its 8x8 output block with only ~3.2x padding waste (vs 15.1x for
  full 136-wide row grams).  C=256 contracts as 2 K=128 matmuls
  accumulated in PSUM.  Chunks ship to DRAM as dense [128,128] bf16
  blocks (4 chunks batched per PSUM bank / copy / DMA); the host slices
  the 81 (dy,dx) diagonal planes with one vectorized fancy-index per
  plane (the shear couples output partition to free offset, which no
  on-chip engine or rectangular DMA access pattern can express).
"""

import ml_dtypes
import numpy as np

import concourse.bass as bass
import concourse.bacc as bacc
import concourse.mybir as mybir
from concourse.bass_utils import run_bass_kernel_spmd
from concourse.tile import TileContext

B, C, H, W = 8, 256, 128, 128
D = 4
ND = 2 * D + 1  # 9 displacements per axis
KT = C // 128  # 2 contraction tiles
WP = W + 2 * D  # 136 padded in2 width (w2 = w + dx space)
WI = W + 16  # 144: in1 padded by 8 on each side (w' = w space, offset 8)
TH = 8  # output h rows per chunk
RH = TH + ND - 1  # 16 in2p rows per chunk (r = h + dy)
TW = 8  # w2 cols per chunk
RW = TW + ND - 1  # 16 in1 cols per chunk (w' = w2 - dx, dx in [0,9))
NHC = H // TH  # 16 h-chunks
NWC = WP // TW  # 17 w2-chunks
GRP = 4  # w-chunks per PSUM bank (4*128 = 512 f32 = one 2KB bank)
# in2 interior row bands (rows in [0,H) interior space; +D in padded row space),
# sized so band i arrives just before the h-chunks that consume it need it.
B2BANDS = [(0, 20), (20, 52), (52, 92), (92, 128)]

_CACHED_NC = None


def _build_nc():
    bf16 = mybir.dt.bfloat16
    f32 = mybir.dt.float32

    nc = bacc.Bacc()
    # in1 interior only, laid out [kt, c, hc, w, h_loc]; the 8-col pads on
    # each side of w' exist only in SBUF and stay uninitialized - they feed
    # exclusively into output slots the host discards.  in2 interior rows
    # with host-padded cols (136 wide); the 4 pad rows top/bottom are
    # memset to zero on-chip (they produce real zero-valued outputs).
    in1_t = nc.declare_dram_parameter("in1_t", [KT, 128, NHC, W, TH], bf16, isOutput=False)
    in2_p = nc.declare_dram_parameter("in2_p", [KT, 128, H, WP], bf16, isOutput=False)
    # dense gram chunks: [hc][m=(w'_loc 16, h_loc 8)][wc][n=(r_loc 16, w2_loc 8)]
    out_g = nc.declare_dram_parameter("out_g", [NHC, 128, NWC, RH * TW], bf16, isOutput=True)

    wgroups = [list(range(g, min(g + GRP, NWC))) for g in range(0, NWC, GRP)]

    with TileContext(nc) as tc:
        with (
            tc.tile_pool(name="bpool", bufs=1) as bpool,
            tc.tile_pool(name="apool", bufs=1) as apool,
            tc.tile_pool(name="spool", bufs=6) as spool,
            tc.tile_pool(name="psum", bufs=8, space="PSUM") as ppool,
        ):
            # whole padded sample resident in SBUF: in2p 72.3KB + in1p 72KB
            # per partition.  Loads are issued in row bands, interleaved so
            # early h-chunks can start while later rows are still in flight.
            b_s = bpool.tile([128, KT, WP, WP], bf16)
            a_s = apool.tile([128, KT, NHC, WI, TH], bf16)
            # zero the 4 pad rows top/bottom of in2p
            for kt in range(KT):
                nc.gpsimd.memset(b_s[:, kt, 0:D, :], 0.0)
                nc.gpsimd.memset(b_s[:, kt, D + H :, :], 0.0)
            # Load DMAs in need-order; the DGE spreads each instruction's
            # packets across all 16 DMA engines and each engine drains its
            # queue in order, so loads complete roughly in issue order at
            # full aggregate bandwidth.  h-chunk hc consumes in2 interior
            # rows [8hc-4, 8hc+12) and in1 group hc//2.
            def load_b(i):
                r0, r1 = B2BANDS[i]
                for kt in range(KT):
                    nc.sync.dma_start(
                        out=b_s[:, kt, D + r0 : D + r1, :],
                        in_=in2_p[kt, :, r0:r1, :],
                    )

            def load_a(g):  # hc pair (2g, 2g+1)
                for kt in range(KT):
                    nc.sync.dma_start(
                        out=a_s[:, kt, 2 * g : 2 * g + 2, 8 : 8 + W, :],
                        in_=in1_t[kt, :, 2 * g : 2 * g + 2],
                    )

            load_b(0)
            load_a(0)
            load_b(1)
            load_a(1)
            load_a(2)
            load_b(2)
            load_a(3)
            load_a(4)
            load_a(5)
            load_b(3)
            load_a(6)
            load_a(7)

            for hc in range(NHC):
                h0 = TH * hc
                st = spool.tile([128, NWC * RH * TW], bf16)
                for wg, wcs in enumerate(wgroups):
                    ps = ppool.tile([128, GRP * RH * TW], f32, name=f"ps{wg}", tag="ps")
                    for j, wc in enumerate(wcs):
                        w0 = TW * wc
                        for kt in range(KT):
                            nc.tensor.matmul(
                                ps[:, 128 * j : 128 * j + 128],
                                a_s[:, kt, hc, w0 : w0 + RW, :],
                                b_s[:, kt, h0 : h0 + RH, w0 : w0 + TW],
                                start=(kt == 0),
                                stop=(kt == KT - 1),
                            )
                    ncol = len(wcs) * RH * TW
                    c0 = GRP * RH * TW * wg
                    nc.any.tensor_copy(st[:, c0 : c0 + ncol], ps[:, :ncol])
                nc.sync.dma_start(
                    out=out_g[hc].rearrange("p w n -> p (w n)"),
                    in_=st,
                )

    # Run the bacc passes (move_matmul_waits_to_ldweights /
    # generate_event_semaphores) that enforce the 1-wait-per-instruction HW
    # constraint.  The native run path calls this inside run_bass_kernel_spmd;
    # the axon/bass2jax path serializes nc without it and walrus then rejects
    # matmuls carrying two sync waits.
    nc.compile()
    return nc


def _get_nc():
    global _CACHED_NC
    if _CACHED_NC is None:
        _CACHED_NC = _build_nc()
    return _CACHED_NC


def _make_in_maps(in1: np.ndarray, in2: np.ndarray):
    in_maps = []
    for b in range(B):
        # [kt, c, hc, h_loc, w] -> [kt, c, hc, w, h_loc]
        a = np.ascontiguousarray(
            in1[b]
            .astype(ml_dtypes.bfloat16)
            .reshape(KT, 128, NHC, TH, W)
            .transpose(0, 1, 2, 4, 3)
        )
        p = np.zeros((KT, 128, H, WP), ml_dtypes.bfloat16)
        p[:, :, :, D : D + W] = in2[b].astype(ml_dtypes.bfloat16).reshape(
            KT, 128, H, W
        )
        in_maps.append({"in1_t": a, "in2_p": p})
    return in_maps


def _extract_band(g: np.ndarray) -> np.ndarray:
    """[NHC, 128, NWC, 128] dense gram chunks -> [81, H, W] cost volume."""
    rf = np.ascontiguousarray(g).astype(np.float32)
    # [hc, w'_loc, h_loc, wc, r_loc, w2_loc]
    r6 = rf.reshape(NHC, RW, TH, NWC, RH, TW)
    hl = np.arange(TH)[:, None]  # (8,1)
    w2 = np.arange(TW)[None, :]  # (1,8)
    out = np.empty((ND * ND, H, W), np.float32)
    for dy in range(ND):
        for dx in range(ND):
            # advanced indices at axes 1,2,4,5 -> result (8,8,NHC,NWC)
            t = r6[:, w2 + 8 - dx, hl, :, hl + dy, w2]
            # (h_loc, w2_loc, hc, wc) -> (hc, h_loc, wc, w2_loc) = (h, w+dx)
            wfull = t.transpose(2, 0, 3, 1).reshape(H, NWC * TW)
            out[dy * ND + dx] = wfull[:, dx : dx + W]
    return out


def kernel(**inputs) -> np.ndarray:
    in1 = np.ascontiguousarray(np.asarray(inputs["in1"], dtype=np.float32))
    in2 = np.ascontiguousarray(np.asarray(inputs["in2"], dtype=np.float32))
    assert in1.shape == (B, C, H, W) and in2.shape == (B, C, H, W)

    nc = _get_nc()
    in_maps = _make_in_maps(in1, in2)
    res = run_bass_kernel_spmd(nc, in_maps, list(range(B)))

    outs = [_extract_band(np.asarray(res.results[b]["out_g"])) for b in range(B)]
    return np.stack(outs).astype(np.float32)
